# revision 11
# baseline (speedup 1.0000x reference)
"""Trainium2 Bass kernel for nn_CrossAttention (8-core data-parallel over batch).

Reference math (per batch b, chunk c):
  en = LayerNorm(e) ; q = en@Wq+bq ; k = h@Wk+bk ; v = h@Wv+bv
  attn = softmax(q@k^T / 8) ; o = attn@v ; out = o@Wo + bo + e

v2 design (vs fp32r baseline):
  - all four 768x768 projections run as fp8(e4m3) DoubleRow matmuls
    (0.5 cyc/row): weights are host-scaled x8 (to keep fp8 mantissa) and
    the 1/8 is folded back at PSUM evacuation
  - attention core (scores / den / AV / bcast) stays bf16 for accuracy
  - O-projection is computed ROW-major (lhsT = oT fp8), so there is no
    transpose-back; the residual (e + bo, folded host-side) is added by the
    PSUM-evacuating scalar_tensor_tensor
  - block-diagonal kbd/vbd tiles are persistent and zero-filled ONCE
  - LayerNorm: bn_stats on DVE, rstd = 1/sqrt via ACT Sqrt + DVE reciprocal,
    normalize on ACT (scale=rstd, bias=-mu*rstd) writing fp8 directly
  - stage A(c+1) / stage B(c) / group phase are emitted INTERLEAVED
    (generators, round-robin) so every engine's in-order queue mixes
    independent work and cross-engine waits hide behind the other stream
  - output store is dispatched from the DVE queue (its producer) so a
    waiting store never blocks the SP load queue
"""

import numpy as np

B, C, N, S, D = 8, 32, 4, 64, 768
NH, DK = 12, 64
R = N * S          # 256 rows per chunk
KO = D // 128      # 6 partition blocks of d
NP = 6             # head pairs
LN_EPS = 1e-5
GROUP = 4          # chunks per h/kv batch group

_prog_cache = {}


def _build(n_chunks, has_qbias=False):
    import concourse.bass as bass
    import concourse.tile as tile
    from concourse import bacc, mybir
    from contextlib import ExitStack

    F32 = mybir.dt.float32
    F32R = mybir.dt.float32r
    BF16 = mybir.dt.bfloat16
    FP8 = mybir.dt.float8e4
    AF = mybir.ActivationFunctionType
    ALU = mybir.AluOpType
    DR = mybir.MatmulPerfMode.DoubleRow

    nc = bacc.Bacc()

    d_e = nc.dram_tensor("e", [n_chunks, R, D], F32, kind="ExternalInput")
    d_h = nc.dram_tensor("h", [n_chunks, S, D], F32R, kind="ExternalInput")
    d_wq = nc.dram_tensor("wq", [KO, 128, D], FP8, kind="ExternalInput")
    d_wk = nc.dram_tensor("wk", [KO, 128, D], FP8, kind="ExternalInput")
    d_wv = nc.dram_tensor("wv", [KO, 128, D], FP8, kind="ExternalInput")
    d_wo = nc.dram_tensor("wo", [KO, 128, D], FP8, kind="ExternalInput")
    d_bqc = nc.dram_tensor("bqc", [128, KO], F32, kind="ExternalInput")
    d_bkc = nc.dram_tensor("bkc", [128, KO], F32, kind="ExternalInput")
    d_e2c = nc.dram_tensor("e2c", [128, NP, NH], BF16, kind="ExternalInput")
    d_rpc = nc.dram_tensor("rpc", [NH, NP, 128], BF16, kind="ExternalInput")
    d_idr = nc.dram_tensor("identr", [128, 128], F32R, kind="ExternalInput")
    d_id8 = nc.dram_tensor("ident8", [128, 128], FP8, kind="ExternalInput")
    d_ones = nc.dram_tensor("onesb", [1, 128], BF16, kind="ExternalInput")
    d_bvrr = nc.dram_tensor("bvrr", [1, D], BF16, kind="ExternalInput")
    d_out = nc.dram_tensor("out", [n_chunks, R, D], F32, kind="ExternalOutput")

    with ExitStack() as ctx:
        tc = ctx.enter_context(tile.TileContext(nc))
        consts = ctx.enter_context(tc.tile_pool(name="consts", bufs=1))
        e_pool = ctx.enter_context(tc.tile_pool(name="e_pool", bufs=2))
        x_pool = ctx.enter_context(tc.tile_pool(name="x_pool", bufs=2))
        xT_pool = ctx.enter_context(tc.tile_pool(name="xT_pool", bufs=2))
        q_pool = ctx.enter_context(tc.tile_pool(name="q_pool", bufs=2))
        exp_pool = ctx.enter_context(tc.tile_pool(name="exp_pool", bufs=2))
        oT_pool = ctx.enter_context(tc.tile_pool(name="oT_pool", bufs=2))
        st_pool = ctx.enter_context(tc.tile_pool(name="st_pool", bufs=2))
        grp_pool = ctx.enter_context(tc.tile_pool(name="grp_pool", bufs=2))
        v2_pool = ctx.enter_context(tc.tile_pool(name="v2_pool", bufs=2))
        ps_t = ctx.enter_context(tc.tile_pool(name="ps_t", bufs=1, space="PSUM"))
        ps_q = ctx.enter_context(tc.tile_pool(name="ps_q", bufs=1, space="PSUM"))
        ps_s = ctx.enter_context(tc.tile_pool(name="ps_s", bufs=3, space="PSUM"))
        ps_o5 = ctx.enter_context(tc.tile_pool(name="ps_o5", bufs=1, space="PSUM"))
        ps_o2 = ctx.enter_context(tc.tile_pool(name="ps_o2", bufs=1, space="PSUM"))

        # ---- constants ----
        wq8 = consts.tile([128, KO, D], FP8)
        wk8 = consts.tile([128, KO, D], FP8)
        wv8 = consts.tile([128, KO, D], FP8)
        wo8 = consts.tile([128, KO, D], FP8)
        nc.sync.dma_start(wq8[:], d_wq[:].rearrange("k p d -> p k d"))
        nc.sync.dma_start(wk8[:], d_wk[:].rearrange("k p d -> p k d"))
        nc.sync.dma_start(wv8[:], d_wv[:].rearrange("k p d -> p k d"))
        nc.sync.dma_start(wo8[:], d_wo[:].rearrange("k p d -> p k d"))
        bqc = consts.tile([128, KO], F32)
        bkc = consts.tile([128, KO], F32)
        nc.sync.dma_start(bqc[:], d_bqc[:])
        nc.sync.dma_start(bkc[:], d_bkc[:])
        e2c = consts.tile([128, NP, NH], BF16)
        rpc = consts.tile([NH, NP, 128], BF16)
        identr = consts.tile([128, 128], F32R)
        ident8 = consts.tile([128, 128], FP8)
        onesb = consts.tile([1, 128], BF16)
        bvrr = consts.tile([1, D], BF16)
        nc.sync.dma_start(e2c[:], d_e2c[:])
        nc.sync.dma_start(rpc[:], d_rpc[:])
        nc.sync.dma_start(identr[:], d_idr[:])
        nc.sync.dma_start(ident8[:], d_id8[:])
        nc.sync.dma_start(onesb[:], d_ones[:])
        nc.sync.dma_start(bvrr[:], d_bvrr[:])

        # persistent block-diagonal tiles: zero-fill ONCE, only diagonal
        # blocks are rewritten (off-diag stays zero forever)
        kbd2 = [consts.tile([128, NP, GROUP, 128], BF16, name=f"kbd{i}")
                for i in range(2)]
        vbd2 = [consts.tile([128, NP, 128], BF16, name=f"vbd{i}")
                for i in range(2)]
        for t_ in kbd2 + vbd2:
            nc.gpsimd.memset(t_[:], 0.0)

        def group_phase(g, hctx):
            # h load (halves), transpose (fp32r), K/V projections (fp8 DR)
            hT4 = grp_pool.tile([128, KO, GROUP * S], FP8, tag="hT4")
            h2s = []
            for hh in range(2):
                h2 = grp_pool.tile([S, 2, D], F32R, tag="h2")
                nc.sync.dma_start(
                    h2[:], d_h[g * GROUP + 2 * hh:g * GROUP + 2 * hh + 2]
                    .rearrange("c j d -> j c d"))
                h2s.append(h2)
                yield
            for cc in range(GROUP):
                h2 = h2s[cc // 2]
                c2 = cc % 2
                pth = ps_t.tile([128, KO, S], F32R, tag="t8", name="pth")
                for i in range(KO):
                    nc.tensor.transpose(
                        pth[:, i, :],
                        h2[:, c2, i * 128:(i + 1) * 128],
                        identr[0:S, 0:S])
                nc.gpsimd.tensor_copy(
                    hT4[:, :, cc * S:(cc + 1) * S], pth[:])
                yield

            # kT in block-diagonal pair layout (persistent tile g%2)
            kbd = kbd2[g % 2]
            pk = ps_q.tile([128, 2, 512], F32, tag="q", name="pk")
            for mo2 in range(KO // 2):
                for m1 in range(2):
                    mo = 2 * mo2 + m1
                    pkh = pk[:, mo2 % 2, m1 * 256:(m1 + 1) * 256]
                    for k2 in range(3):
                        nc.tensor.matmul(
                            pkh,
                            wk8[:, 2 * k2:2 * k2 + 2, mo * 128:(mo + 1) * 128],
                            hT4[:, 2 * k2:2 * k2 + 2, :],
                            start=(k2 == 0), stop=(k2 == 2), perf_mode=DR)
                    pkv = pkh.rearrange("p (c j) -> p c j", c=GROUP)
                    nc.gpsimd.tensor_scalar(
                        out=kbd[0:64, mo, :, 0:S], in0=pkv[0:64],
                        scalar1=0.125, scalar2=bkc[0:64, mo:mo + 1],
                        op0=ALU.mult, op1=ALU.add)
                    nc.gpsimd.tensor_scalar(
                        out=kbd[64:128, mo, :, S:128], in0=pkv[64:128],
                        scalar1=0.125, scalar2=bkc[64:128, mo:mo + 1],
                        op0=ALU.mult, op1=ALU.add)
                yield

            v2 = []
            for st in range(GROUP // 2):
                v2t = v2_pool.tile([128, D], BF16, tag="v2")
                pv5 = ps_o5.tile([128, 512], F32, tag="o5", name="pv5")
                pv2 = ps_o2.tile([128, 256], F32, tag="o2", name="pv2")
                for k2 in range(3):
                    nc.tensor.matmul(
                        pv5[:],
                        hT4[:, 2 * k2:2 * k2 + 2, st * 128:(st + 1) * 128],
                        wv8[:, 2 * k2:2 * k2 + 2, 0:512],
                        start=(k2 == 0), stop=False, perf_mode=DR)
                    nc.tensor.matmul(
                        pv2[:],
                        hT4[:, 2 * k2:2 * k2 + 2, st * 128:(st + 1) * 128],
                        wv8[:, 2 * k2:2 * k2 + 2, 512:768],
                        start=(k2 == 0), stop=False, perf_mode=DR)
                nc.tensor.matmul(
                    pv5[:], onesb[:], bvrr[:, 0:512], start=False, stop=True)
                nc.tensor.matmul(
                    pv2[:], onesb[:], bvrr[:, 512:768], start=False, stop=True)
                nc.vector.tensor_scalar(
                    out=v2t[:, 0:512], in0=pv5[:], scalar1=0.125,
                    scalar2=None, op0=ALU.mult)
                nc.vector.tensor_scalar(
                    out=v2t[:, 512:768], in0=pv2[:], scalar1=0.125,
                    scalar2=None, op0=ALU.mult)
                v2.append(v2t)
                hctx["v2"] = v2
                yield

        def stage_a(c, hctx, actx):
            cc = c % GROUP
            # ---- load e in halves (residual has bo folded host-side) ----
            e_sb = e_pool.tile([128, 2, D], F32, tag="e")
            stats = st_pool.tile([128, 2, 3, 6], F32, tag="stats")
            mv = st_pool.tile([128, 2, 2], F32, tag="mv")
            for t in range(2):
                nc.sync.dma_start(
                    e_sb[:, t, :], d_e[c, t * 128:(t + 1) * 128, :])
                esl = e_sb[:, t, :].rearrange("p (s f) -> p s f", s=3)
                for sg in range(3):
                    nc.vector.bn_stats(stats[:, t, sg, :], esl[:, sg, :])
                nc.vector.bn_aggr(mv[:, t, :], stats[:, t, :, :])
                yield

            # rstd = 1/sqrt(var+eps): ACT Sqrt + DVE reciprocal
            rstd = st_pool.tile([128, 2], F32, tag="rstd")
            nmr = st_pool.tile([128, 2], F32, tag="nmr")
            v1 = st_pool.tile([128, 2], F32, tag="v1")
            sq = st_pool.tile([128, 2], F32, tag="sq")
            nc.vector.tensor_scalar(
                out=v1[:], in0=mv[:, :, 1], scalar1=float(LN_EPS), scalar2=None,
                op0=ALU.add)
            nc.scalar.activation(sq[:], v1[:], AF.Sqrt)
            nc.vector.reciprocal(rstd[:], sq[:])
            nc.vector.scalar_tensor_tensor(
                out=nmr[:], in0=mv[:, :, 0], scalar=-1.0, in1=rstd[:],
                op0=ALU.mult, op1=ALU.mult)
            yield

            # ---- normalize on ACT -> x8 (fp8); transpose; evac on Pool ----
            x8 = x_pool.tile([128, 2, D], FP8, tag="x")
            xT8 = xT_pool.tile([128, KO, R], FP8, tag="xT")
            for t in range(2):
                nc.scalar.activation(
                    x8[:, t, :], e_sb[:, t, :], AF.Identity,
                    bias=nmr[:, t:t + 1], scale=rstd[:, t:t + 1])
                pt = ps_t.tile([128, KO, 128], FP8, tag="t8", name="pt")
                for i in range(KO):
                    nc.tensor.transpose(
                        pt[:, i, :], x8[:, t, i * 128:(i + 1) * 128],
                        ident8[:])
                nc.gpsimd.tensor_copy(
                    xT8[:, :, t * 128:(t + 1) * 128], pt[:])
                yield

            # ---- Q projection (fp8 DR), evac on ACT with 1/64 (+bias) ----
            qT = q_pool.tile([128, KO, R], BF16, tag="qT")
            pq = ps_q.tile([128, 2, 512], F32, tag="q", name="pq")
            for mo2 in range(KO // 2):
                for m1 in range(2):
                    mo = 2 * mo2 + m1
                    for k2 in range(3):
                        nc.tensor.matmul(
                            pq[:, mo2 % 2, m1 * 256:(m1 + 1) * 256],
                            wq8[:, 2 * k2:2 * k2 + 2, mo * 128:(mo + 1) * 128],
                            xT8[:, 2 * k2:2 * k2 + 2, :],
                            start=(k2 == 0), stop=(k2 == 2), perf_mode=DR)
                if has_qbias:
                    for m1 in range(2):
                        mo = 2 * mo2 + m1
                        nc.scalar.activation(
                            qT[:, mo, :],
                            pq[:, mo2 % 2, m1 * 256:(m1 + 1) * 256],
                            AF.Identity,
                            bias=bqc[:, mo:mo + 1], scale=0.015625)
                else:
                    nc.scalar.activation(
                        qT[:, 2 * mo2:2 * mo2 + 2, :],
                        pq[:, mo2 % 2, :].rearrange("p (a b) -> p a b", a=2),
                        AF.Identity, bias=0.0, scale=0.015625)
                yield

            # ---- v in block-diagonal pair layout (persistent tile cc%2) ----
            while "v2" not in hctx or len(hctx["v2"]) <= cc // 2:
                yield
            v2t = hctx["v2"][cc // 2]
            pa = 64 * (cc % 2)
            vbd = vbd2[cc % 2]
            v2v = v2t[pa:pa + 64, :].rearrange(
                "p (np two dk) -> p np two dk", np=NP, two=2)
            if cc % 2 == 0:
                nc.vector.tensor_copy(vbd[0:64, :, 0:DK], v2v[:, :, 0, :])
                nc.gpsimd.tensor_copy(vbd[64:128, :, DK:128], v2v[:, :, 1, :])
            else:
                nc.gpsimd.tensor_copy(vbd[0:64, :, 0:DK], v2v[:, :, 0, :])
                nc.vector.tensor_copy(vbd[64:128, :, DK:128], v2v[:, :, 1, :])
            actx.update(c=c, e_sb=e_sb, qT=qT, vbd=vbd)
            yield

        def stage_b(actx, hctx):
            c = actx["c"]
            e_sb, qT, vbd = actx["e_sb"], actx["qT"], actx["vbd"]
            kbd = kbd2[(c // GROUP) % 2]
            cc = c % GROUP

            # ---- scores (bf16) + exp on ACT, den interleaved ----
            expT = exp_pool.tile([128, NP, R], BF16, tag="expT")
            pden_t = None
            for p2 in range(0, NP, 2):
                pscr = ps_s.tile([128, 2, R], F32, tag="s")
                for i in range(2):
                    nc.tensor.matmul(
                        pscr[:, i, :], kbd[:, p2 + i, cc, :], qT[:, p2 + i, :],
                        start=True, stop=True)
                nc.scalar.activation(
                    expT[:, p2:p2 + 2, :], pscr[:], AF.Exp, bias=0.0, scale=1.0)
                if p2 == 0:
                    pden_t = ps_s.tile([128, R], F32, tag="s", name="pden_t")
                if p2 > 0:
                    for p in (p2 - 2, p2 - 1):
                        nc.tensor.matmul(
                            pden_t[0:NH, :], e2c[:, p, :], expT[:, p, :],
                            start=(p == 0), stop=False,
                            skip_group_check=True)
                yield
            for p in (NP - 2, NP - 1):
                nc.tensor.matmul(
                    pden_t[0:NH, :], e2c[:, p, :], expT[:, p, :],
                    start=False, stop=(p == NP - 1), skip_group_check=True)
            recip = st_pool.tile([NH, R], BF16, tag="recip")
            with nc.allow_low_precision(reason="bf16 softmax denom"):
                nc.vector.reciprocal(recip[:], pden_t[0:NH, :])
            yield

            # ---- AV + recip broadcast + normalize into oT (fp8) ----
            oT = oT_pool.tile([128, KO, R], FP8, tag="oT")
            for p2 in range(0, NP, 2):
                pav = ps_s.tile([128, 2, R], F32, tag="s", name="pav")
                pbc = ps_s.tile([128, 2, R], F32, tag="s", name="pbc")
                for i in range(2):
                    nc.tensor.matmul(
                        pav[:, i, :], vbd[:, p2 + i, :], expT[:, p2 + i, :],
                        start=True, stop=True)
                    nc.tensor.matmul(
                        pbc[:, i, :], rpc[:, p2 + i, :], recip[:],
                        start=True, stop=True)
                with nc.allow_low_precision(reason="fp8 oT"):
                    nc.vector.tensor_tensor(
                        out=oT[:, p2:p2 + 2, :], in0=pav[:], in1=pbc[:],
                        op=ALU.mult)
                yield

            # ---- O projection ROW-major (fp8 DR) + residual evac (DVE) ----
            for rb in range(2):
                po5 = ps_o5.tile([128, 512], F32, tag="o5", name="po5")
                po2 = ps_o2.tile([128, 256], F32, tag="o2", name="po2")
                for k2 in range(3):
                    nc.tensor.matmul(
                        po5[:],
                        oT[:, 2 * k2:2 * k2 + 2, rb * 128:(rb + 1) * 128],
                        wo8[:, 2 * k2:2 * k2 + 2, 0:512],
                        start=(k2 == 0), stop=(k2 == 2), perf_mode=DR)
                    nc.tensor.matmul(
                        po2[:],
                        oT[:, 2 * k2:2 * k2 + 2, rb * 128:(rb + 1) * 128],
                        wo8[:, 2 * k2:2 * k2 + 2, 512:768],
                        start=(k2 == 0), stop=(k2 == 2), perf_mode=DR)
                nc.vector.scalar_tensor_tensor(
                    out=e_sb[:, rb, 0:512], in0=po5[:], scalar=0.125,
                    in1=e_sb[:, rb, 0:512], op0=ALU.mult, op1=ALU.add)
                nc.vector.scalar_tensor_tensor(
                    out=e_sb[:, rb, 512:768], in0=po2[:], scalar=0.125,
                    in1=e_sb[:, rb, 512:768], op0=ALU.mult, op1=ALU.add)
                yield
            nc.sync.dma_start(
                d_out[c].rearrange("(t p) d -> p t d", p=128), e_sb[:])
            yield

        def interleave(gens):
            gens = [g for g in gens if g is not None]
            while gens:
                alive = []
                for g_ in gens:
                    try:
                        next(g_)
                        alive.append(g_)
                    except StopIteration:
                        pass
                gens = alive

        # ---- pipelined driver: group(g) / A(c) / B(c-1) interleaved ----
        n_groups = n_chunks // GROUP
        pend = None
        for g in range(n_groups):
            hctx = {}
            ggen = group_phase(g, hctx)
            for cc in range(GROUP):
                c = g * GROUP + cc
                actx = {}
                agen = stage_a(c, hctx, actx)
                bgen = stage_b(*pend) if pend is not None else None
                interleave([ggen if cc == 0 else None, agen, bgen])
                pend = (actx, hctx)
        interleave([stage_b(*pend)])

    nc.compile()
    return nc


def _prep_consts(Wq, bq, Wk, bk, Wv, bv, Wo, bo, ln_g, ln_b):
    import ml_dtypes
    FP8 = ml_dtypes.float8_e4m3
    BF16 = ml_dtypes.bfloat16

    Wq_l = ln_g[:, None] * Wq
    bq_eff = (ln_b @ Wq + bq) * 0.125

    def wl8(w):
        return np.ascontiguousarray(
            (8.0 * w).reshape(KO, 128, D).astype(FP8))

    e2c = np.zeros((128, NP, NH), dtype=np.float32)
    for p in range(NP):
        e2c[0:64, p, 2 * p] = 1.0
        e2c[64:128, p, 2 * p + 1] = 1.0
    rpc = np.zeros((NH, NP, 128), dtype=np.float32)
    for p in range(NP):
        rpc[2 * p, p, 0:64] = 1.0
        rpc[2 * p + 1, p, 64:128] = 1.0

    return {
        "wq": wl8(Wq_l), "wk": wl8(Wk), "wv": wl8(Wv), "wo": wl8(Wo),
        "e2c": e2c.astype(BF16), "rpc": rpc.astype(BF16),
        "onesb": np.ones((1, 128), dtype=np.float32).astype(BF16),
        "bqc": np.ascontiguousarray(bq_eff.reshape(KO, 128).T, dtype=np.float32),
        "bkc": np.ascontiguousarray(bk.reshape(KO, 128).T, dtype=np.float32),
        "bvrr": np.ascontiguousarray((8.0 * bv).reshape(1, D)).astype(BF16),
        "identr": np.eye(128, dtype=np.float32),
        "ident8": np.eye(128, dtype=np.float32).astype(FP8),
    }


def kernel(e, h, Wq, bq, Wk, bk, Wv, bv, Wo, bo, ln_g, ln_b):
    from concourse.bass_utils import run_bass_kernel_spmd

    e = np.asarray(e, dtype=np.float32)
    h = np.asarray(h, dtype=np.float32)
    bo = np.asarray(bo, dtype=np.float32)
    n_chunks = e.shape[1]

    bq_eff = np.asarray(ln_b, np.float32) @ np.asarray(Wq, np.float32) \
        + np.asarray(bq, np.float32)
    has_qbias = bool(np.any(bq_eff != 0.0))
    key = (n_chunks, has_qbias)
    if key not in _prog_cache:
        _prog_cache[key] = _build(n_chunks, has_qbias)
        _prog_cache[n_chunks] = _prog_cache[key]  # for test.py's TimelineSim
    nc = _prog_cache[key]

    consts = _prep_consts(
        np.asarray(Wq, np.float32), np.asarray(bq, np.float32),
        np.asarray(Wk, np.float32), np.asarray(bk, np.float32),
        np.asarray(Wv, np.float32), np.asarray(bv, np.float32),
        np.asarray(Wo, np.float32), np.asarray(bo, np.float32),
        np.asarray(ln_g, np.float32), np.asarray(ln_b, np.float32))

    if np.any(bo):
        e = e + bo  # fold output bias into the residual

    in_maps = []
    for b in range(B):
        m = dict(consts)
        m["e"] = np.ascontiguousarray(e[b].reshape(n_chunks, R, D))
        m["h"] = np.ascontiguousarray(h[b])
        in_maps.append(m)

    res = run_bass_kernel_spmd(nc, in_maps, core_ids=list(range(B)))
    out = np.stack([r["out"] for r in res.results], axis=0)
    return out.reshape(B, n_chunks, N, S, D)


# revision 14
# speedup vs baseline: 1.6485x; 1.6485x over previous
"""Trainium2 Bass kernel for nn_CrossAttention (8-core data-parallel over batch).

Reference math (per batch b, chunk c):
  en = LayerNorm(e) ; q = en@Wq+bq ; k = h@Wk+bk ; v = h@Wv+bv
  attn = softmax(q@k^T / 8) ; o = attn@v ; out = o@Wo + bo + e

v3 design:
  - all four 768x768 projections are fp8(e4m3) DoubleRow matmuls
    (0.5 cyc/row): weights host-scaled x8 (fp8 mantissa range), 1/8 folded
    back at PSUM evacuation; attention core (scores/den/AV/bcast) is bf16
  - O-projection computed ROW-major (lhsT = oT fp8): no transpose-back;
    residual (e + bo folded host-side) added by the evacuating
    scalar_tensor_tensor
  - block-diagonal kbd/vbd tiles are persistent, zero-filled once
  - 6-stage software pipeline, emitted interleaved per slot (oldest first):
      B3(c-5) O+resid+store | B2(c-4) AV/bcast/oT | B1(c-3) scores/exp/den
      | M(c-2) Q-proj+vbd | A2(c-1) norm+transpose | A1(c) load+stats
    plus the group phase (h transpose, K/V projections) at group starts
"""

import numpy as np

B, C, N, S, D = 8, 32, 4, 64, 768
NH, DK = 12, 64
R = N * S          # 256 rows per chunk
KO = D // 128      # 6 partition blocks of d
NP = 6             # head pairs
LN_EPS = 1e-5
GROUP = 4          # chunks per h/kv batch group

_prog_cache = {}


def _build(n_chunks):
    import concourse.bass as bass
    import concourse.tile as tile
    from concourse import bacc, mybir
    from contextlib import ExitStack

    F32 = mybir.dt.float32
    F32R = mybir.dt.float32r
    BF16 = mybir.dt.bfloat16
    FP8 = mybir.dt.float8e4
    I32 = mybir.dt.int32
    AF = mybir.ActivationFunctionType
    ALU = mybir.AluOpType
    DR = mybir.MatmulPerfMode.DoubleRow

    nc = bacc.Bacc()

    d_e = nc.dram_tensor("e", [n_chunks, R, D], F32, kind="ExternalInput")
    d_h = nc.dram_tensor("h", [n_chunks, S, D], F32R, kind="ExternalInput")
    d_wq = nc.dram_tensor("wq", [KO, 128, D], FP8, kind="ExternalInput")
    d_wk = nc.dram_tensor("wk", [KO, 128, D], FP8, kind="ExternalInput")
    d_wv = nc.dram_tensor("wv", [KO, 128, D], FP8, kind="ExternalInput")
    d_wo = nc.dram_tensor("wo", [KO, 128, D], FP8, kind="ExternalInput")
    d_bqc = nc.dram_tensor("bqc", [128, KO], F32, kind="ExternalInput")
    d_bkc = nc.dram_tensor("bkc", [128, KO], F32, kind="ExternalInput")
    d_e2c = nc.dram_tensor("e2c", [128, NP, NH], BF16, kind="ExternalInput")
    d_rpc = nc.dram_tensor("rpc", [NH, NP, 128], BF16, kind="ExternalInput")
    d_idr = nc.dram_tensor("identr", [128, 128], F32R, kind="ExternalInput")
    d_id8 = nc.dram_tensor("ident8", [128, 128], FP8, kind="ExternalInput")
    d_ones = nc.dram_tensor("onesb", [1, 128], BF16, kind="ExternalInput")
    d_bvrr = nc.dram_tensor("bvrr", [1, D], BF16, kind="ExternalInput")
    d_out = nc.dram_tensor("out", [n_chunks, R, D], F32, kind="ExternalOutput")

    with ExitStack() as ctx:
        tc = ctx.enter_context(tile.TileContext(nc))
        consts = ctx.enter_context(tc.tile_pool(name="consts", bufs=1))
        e_pool = ctx.enter_context(tc.tile_pool(name="e_pool", bufs=6))
        x_pool = ctx.enter_context(tc.tile_pool(name="x_pool", bufs=2))
        xT_pool = ctx.enter_context(tc.tile_pool(name="xT_pool", bufs=2))
        q_pool = ctx.enter_context(tc.tile_pool(name="q_pool", bufs=2))
        exp_pool = ctx.enter_context(tc.tile_pool(name="exp_pool", bufs=2))
        oT_pool = ctx.enter_context(tc.tile_pool(name="oT_pool", bufs=2))
        st_pool = ctx.enter_context(tc.tile_pool(name="st_pool", bufs=2))
        grp_pool = ctx.enter_context(tc.tile_pool(name="grp_pool", bufs=2))
        v2_pool = ctx.enter_context(tc.tile_pool(name="v2_pool", bufs=2))
        ps_t = ctx.enter_context(tc.tile_pool(name="ps_t", bufs=1, space="PSUM"))
        ps_q = ctx.enter_context(tc.tile_pool(name="ps_q", bufs=1, space="PSUM"))
        ps_s = ctx.enter_context(tc.tile_pool(name="ps_s", bufs=2, space="PSUM"))
        ps_ab = ctx.enter_context(tc.tile_pool(name="ps_ab", bufs=1, space="PSUM"))
        ps_o5 = ctx.enter_context(tc.tile_pool(name="ps_o5", bufs=1, space="PSUM"))
        ps_o2 = ctx.enter_context(tc.tile_pool(name="ps_o2", bufs=1, space="PSUM"))

        # ---- constants ----
        wq8 = consts.tile([128, KO, D], FP8)
        wk8 = consts.tile([128, KO, D], FP8)
        wv8 = consts.tile([128, KO, D], FP8)
        wo8 = consts.tile([128, KO, D], FP8)
        nc.sync.dma_start(wq8[:], d_wq[:].rearrange("k p d -> p k d"))
        nc.sync.dma_start(wk8[:], d_wk[:].rearrange("k p d -> p k d"))
        nc.sync.dma_start(wv8[:], d_wv[:].rearrange("k p d -> p k d"))
        nc.sync.dma_start(wo8[:], d_wo[:].rearrange("k p d -> p k d"))
        bqc = consts.tile([128, KO], F32)
        bkc = consts.tile([128, KO], F32)
        nc.sync.dma_start(bqc[:], d_bqc[:])
        nc.sync.dma_start(bkc[:], d_bkc[:])
        e2c = consts.tile([128, NP, NH], BF16)
        rpc = consts.tile([NH, NP, 128], BF16)
        identr = consts.tile([128, 128], F32R)
        ident8 = consts.tile([128, 128], FP8)
        onesb = consts.tile([1, 128], BF16)
        bvrr = consts.tile([1, D], BF16)
        nc.sync.dma_start(e2c[:], d_e2c[:])
        nc.sync.dma_start(rpc[:], d_rpc[:])
        nc.sync.dma_start(identr[:], d_idr[:])
        nc.sync.dma_start(ident8[:], d_id8[:])
        nc.sync.dma_start(onesb[:], d_ones[:])
        nc.sync.dma_start(bvrr[:], d_bvrr[:])

        # persistent block-diagonal tiles: zero-fill ONCE, only diagonal
        # blocks are rewritten (off-diag stays zero forever)
        kbd2 = [consts.tile([128, NP, GROUP, 128], BF16, name=f"kbd{i}")
                for i in range(2)]
        vbd4 = [consts.tile([128, NP, 128], BF16, name=f"vbd{i}")
                for i in range(4)]
        for t_ in kbd2 + vbd4:
            nc.gpsimd.memset(t_[:], 0.0)

        def group_phase(g, hctx):
            # h load (halves), transpose (fp32r), K/V projections (fp8 DR)
            hT4 = grp_pool.tile([128, KO, GROUP * S], FP8, tag="hT4")
            h2s = []
            for hh in range(2):
                h2 = grp_pool.tile([S, 2, D], F32R, tag="h2")
                nc.sync.dma_start(
                    h2[:], d_h[g * GROUP + 2 * hh:g * GROUP + 2 * hh + 2]
                    .rearrange("c j d -> j c d"))
                h2s.append(h2)
                yield
            for cc in range(GROUP):
                h2 = h2s[cc // 2]
                c2 = cc % 2
                pth = ps_t.tile([128, KO, S], F32R, tag="t8", name="pth")
                for i in range(KO):
                    nc.tensor.transpose(
                        pth[:, i, :],
                        h2[:, c2, i * 128:(i + 1) * 128],
                        identr[0:S, 0:S])
                nc.gpsimd.tensor_copy(
                    hT4[:, :, cc * S:(cc + 1) * S], pth[:])
                yield

            # kT in block-diagonal pair layout (persistent tile g%2)
            kbd = kbd2[g % 2]
            pk = ps_q.tile([128, 2, 256], F32, tag="q", name="pk")
            for mo in range(KO):
                pkh = pk[:, mo % 2, :]
                for k2 in range(3):
                    nc.tensor.matmul(
                        pkh,
                        wk8[:, 2 * k2:2 * k2 + 2, mo * 128:(mo + 1) * 128],
                        hT4[:, 2 * k2:2 * k2 + 2, :],
                        start=(k2 == 0), stop=(k2 == 2), perf_mode=DR)
                pkv = pkh.rearrange("p (c j) -> p c j", c=GROUP)
                nc.gpsimd.tensor_scalar(
                    out=kbd[0:64, mo, :, 0:S], in0=pkv[0:64],
                    scalar1=0.125, scalar2=bkc[0:64, mo:mo + 1],
                    op0=ALU.mult, op1=ALU.add)
                nc.gpsimd.tensor_scalar(
                    out=kbd[64:128, mo, :, S:128], in0=pkv[64:128],
                    scalar1=0.125, scalar2=bkc[64:128, mo:mo + 1],
                    op0=ALU.mult, op1=ALU.add)
                if mo % 2 == 1:
                    yield

            v2 = []
            for st in range(GROUP // 2):
                v2t = v2_pool.tile([128, D], BF16, tag="v2")
                pv5 = ps_o5.tile([128, 512], F32, tag="o5", name="pv5")
                pv2 = ps_o2.tile([128, 256], F32, tag="o2", name="pv2")
                for k2 in range(3):
                    nc.tensor.matmul(
                        pv5[:],
                        hT4[:, 2 * k2:2 * k2 + 2, st * 128:(st + 1) * 128],
                        wv8[:, 2 * k2:2 * k2 + 2, 0:512],
                        start=(k2 == 0), stop=False, perf_mode=DR)
                    nc.tensor.matmul(
                        pv2[:],
                        hT4[:, 2 * k2:2 * k2 + 2, st * 128:(st + 1) * 128],
                        wv8[:, 2 * k2:2 * k2 + 2, 512:768],
                        start=(k2 == 0), stop=False, perf_mode=DR)
                nc.tensor.matmul(
                    pv5[:], onesb[:], bvrr[:, 0:512], start=False, stop=True)
                nc.tensor.matmul(
                    pv2[:], onesb[:], bvrr[:, 512:768], start=False, stop=True)
                nc.vector.tensor_scalar(
                    out=v2t[:, 0:512], in0=pv5[:], scalar1=0.125,
                    scalar2=None, op0=ALU.mult)
                nc.vector.tensor_scalar(
                    out=v2t[:, 512:768], in0=pv2[:], scalar1=0.125,
                    scalar2=None, op0=ALU.mult)
                v2.append(v2t)
                hctx["v2"] = v2
                yield

        def stage_a1(c, cx):
            # load e in halves + LN stats; rstd via quake-rsqrt (1 Newton)
            e_sb = e_pool.tile([128, 2, D], F32, tag="e")
            stats = st_pool.tile([128, 2, 2, 6], F32, tag="stats")
            mv = st_pool.tile([128, 2, 2], F32, tag="mv")
            for t in range(2):
                nc.sync.dma_start(
                    e_sb[:, t, :], d_e[c, t * 128:(t + 1) * 128, :])
                nc.vector.bn_stats(stats[:, t, 0, :], e_sb[:, t, 0:512])
                nc.vector.bn_stats(stats[:, t, 1, :], e_sb[:, t, 512:768])
                nc.vector.bn_aggr(mv[:, t, :], stats[:, t, :, :])
                yield
            rstd = st_pool.tile([128, 2], F32, tag="rstd")
            nmr = st_pool.tile([128, 2], F32, tag="nmr")
            v1 = st_pool.tile([128, 2], F32, tag="v1")
            y = st_pool.tile([128, 2], F32, tag="y")
            tmp = st_pool.tile([128, 2], F32, tag="tmp")
            nc.vector.tensor_scalar(
                out=v1[:], in0=mv[:, :, 1], scalar1=float(LN_EPS), scalar2=None,
                op0=ALU.add)
            nc.vector.tensor_scalar(
                out=y[:].bitcast(I32), in0=v1[:].bitcast(I32), scalar1=1,
                scalar2=None, op0=ALU.logical_shift_right)
            nc.vector.tensor_scalar(
                out=y[:].bitcast(I32), in0=y[:].bitcast(I32), scalar1=-1,
                scalar2=0x5F3759DF, op0=ALU.mult, op1=ALU.add)
            nc.vector.tensor_tensor(out=tmp[:], in0=y[:], in1=y[:], op=ALU.mult)
            nc.vector.tensor_tensor(out=tmp[:], in0=tmp[:], in1=v1[:], op=ALU.mult)
            nc.vector.tensor_scalar(
                out=tmp[:], in0=tmp[:], scalar1=-0.5, scalar2=1.5,
                op0=ALU.mult, op1=ALU.add)
            nc.vector.tensor_tensor(out=rstd[:], in0=y[:], in1=tmp[:], op=ALU.mult)
            nc.vector.scalar_tensor_tensor(
                out=nmr[:], in0=mv[:, :, 0], scalar=-1.0, in1=rstd[:],
                op0=ALU.mult, op1=ALU.mult)
            cx.update(e_sb=e_sb, rstd=rstd, nmr=nmr)
            yield

        def stage_a2(c, cx):
            # normalize on ACT -> x8 (fp8); transpose on PE; evac on Pool
            e_sb, rstd, nmr = cx["e_sb"], cx["rstd"], cx["nmr"]
            x8 = x_pool.tile([128, 2, D], FP8, tag="x")
            xT8 = xT_pool.tile([128, KO, R], FP8, tag="xT")
            for t in range(2):
                nc.scalar.activation(
                    x8[:, t, :], e_sb[:, t, :], AF.Identity,
                    bias=nmr[:, t:t + 1], scale=rstd[:, t:t + 1])
                pt = ps_t.tile([128, KO, 128], FP8, tag="t8", name="pt")
                for i in range(KO):
                    nc.tensor.transpose(
                        pt[:, i, :], x8[:, t, i * 128:(i + 1) * 128],
                        ident8[:])
                nc.gpsimd.tensor_copy(
                    xT8[:, :, t * 128:(t + 1) * 128], pt[:])
                yield
            cx.update(xT8=xT8)

        def stage_m(c, cx, hctx):
            # Q projection (fp8 DR), evac on ACT with 1/64 + bias
            xT8 = cx["xT8"]
            qT = q_pool.tile([128, KO, R], BF16, tag="qT")
            pq = ps_q.tile([128, 2, 256], F32, tag="q", name="pq")
            for mo in range(KO):
                pqh = pq[:, mo % 2, :]
                for k2 in range(3):
                    nc.tensor.matmul(
                        pqh,
                        wq8[:, 2 * k2:2 * k2 + 2, mo * 128:(mo + 1) * 128],
                        xT8[:, 2 * k2:2 * k2 + 2, :],
                        start=(k2 == 0), stop=(k2 == 2), perf_mode=DR)
                nc.scalar.activation(
                    qT[:, mo, :], pqh, AF.Identity,
                    bias=bqc[:, mo:mo + 1], scale=0.015625)
                if mo % 2 == 1:
                    yield

            # v in block-diagonal pair layout (persistent tile c%4)
            cc = c % GROUP
            while "v2" not in hctx or len(hctx["v2"]) <= cc // 2:
                yield
            v2t = hctx["v2"][cc // 2]
            pa = 64 * (cc % 2)
            vbd = vbd4[c % 4]
            v2v = v2t[pa:pa + 64, :].rearrange(
                "p (np two dk) -> p np two dk", np=NP, two=2)
            if cc % 2 == 0:
                nc.vector.tensor_copy(vbd[0:64, :, 0:DK], v2v[:, :, 0, :])
                nc.gpsimd.tensor_copy(vbd[64:128, :, DK:128], v2v[:, :, 1, :])
            else:
                nc.gpsimd.tensor_copy(vbd[0:64, :, 0:DK], v2v[:, :, 0, :])
                nc.vector.tensor_copy(vbd[64:128, :, DK:128], v2v[:, :, 1, :])
            cx.update(qT=qT, vbd=vbd)
            yield

        def stage_b1(c, cx):
            # scores (bf16) + exp on ACT, den interleaved, reciprocal
            qT = cx["qT"]
            kbd = kbd2[(c // GROUP) % 2]
            cc = c % GROUP
            expT = exp_pool.tile([128, NP, R], BF16, tag="expT")
            for p2 in range(0, NP, 2):
                pscr = ps_s.tile([128, 2, R], F32, tag="s")
                for i in range(2):
                    nc.tensor.matmul(
                        pscr[:, i, :], kbd[:, p2 + i, cc, :], qT[:, p2 + i, :],
                        start=True, stop=True)
                nc.scalar.activation(
                    expT[:, p2:p2 + 2, :], pscr[:], AF.Exp, bias=0.0, scale=1.0)
                yield
            pden_t = ps_s.tile([128, R], F32, tag="s", name="pden_t")
            for p in range(NP):
                nc.tensor.matmul(
                    pden_t[0:NH, :], e2c[:, p, :], expT[:, p, :],
                    start=(p == 0), stop=(p == NP - 1), skip_group_check=True)
            recip = st_pool.tile([NH, R], BF16, tag="recip")
            with nc.allow_low_precision(reason="bf16 softmax denom"):
                nc.vector.reciprocal(recip[:], pden_t[0:NH, :])
            cx.update(expT=expT, recip=recip)
            yield

        def stage_b2(c, cx):
            # AV + recip broadcast + normalize into oT (fp8)
            expT, recip, vbd = cx["expT"], cx["recip"], cx["vbd"]
            oT = oT_pool.tile([128, KO, R], FP8, tag="oT")
            for p2 in range(0, NP, 2):
                ab = ps_ab.tile([128, 2, 2, R], F32, tag="ab")
                for i in range(2):
                    nc.tensor.matmul(
                        ab[:, 0, i, :], vbd[:, p2 + i, :], expT[:, p2 + i, :],
                        start=True, stop=True)
                    nc.tensor.matmul(
                        ab[:, 1, i, :], rpc[:, p2 + i, :], recip[:],
                        start=True, stop=True)
                with nc.allow_low_precision(reason="fp8 oT"):
                    nc.vector.tensor_tensor(
                        out=oT[:, p2:p2 + 2, :], in0=ab[:, 0, :, :],
                        in1=ab[:, 1, :, :], op=ALU.mult)
                yield
            cx.update(oT=oT)

        def stage_b3(c, cx):
            # O projection ROW-major (fp8 DR) + residual evac + store
            e_sb, oT = cx["e_sb"], cx["oT"]
            for rb in range(2):
                po5 = ps_o5.tile([128, 512], F32, tag="o5", name="po5")
                po2 = ps_o2.tile([128, 256], F32, tag="o2", name="po2")
                for k2 in range(3):
                    nc.tensor.matmul(
                        po5[:],
                        oT[:, 2 * k2:2 * k2 + 2, rb * 128:(rb + 1) * 128],
                        wo8[:, 2 * k2:2 * k2 + 2, 0:512],
                        start=(k2 == 0), stop=(k2 == 2), perf_mode=DR)
                    nc.tensor.matmul(
                        po2[:],
                        oT[:, 2 * k2:2 * k2 + 2, rb * 128:(rb + 1) * 128],
                        wo8[:, 2 * k2:2 * k2 + 2, 512:768],
                        start=(k2 == 0), stop=(k2 == 2), perf_mode=DR)
                nc.vector.scalar_tensor_tensor(
                    out=e_sb[:, rb, 0:512], in0=po5[:], scalar=0.125,
                    in1=e_sb[:, rb, 0:512], op0=ALU.mult, op1=ALU.add)
                nc.gpsimd.scalar_tensor_tensor(
                    out=e_sb[:, rb, 512:768], in0=po2[:], scalar=0.125,
                    in1=e_sb[:, rb, 512:768], op0=ALU.mult, op1=ALU.add)
                yield
            nc.sync.dma_start(
                d_out[c].rearrange("(t p) d -> p t d", p=128), e_sb[:])
            yield

        def interleave(gens):
            gens = [g_ for g_ in gens if g_ is not None]
            while gens:
                alive = []
                for g_ in gens:
                    try:
                        next(g_)
                        alive.append(g_)
                    except StopIteration:
                        pass
                gens = alive

        # ---- slot-based pipelined driver ----
        n_groups = n_chunks // GROUP
        cxs = [dict() for _ in range(n_chunks)]
        hctxs = [dict() for _ in range(n_groups)]

        def hx(c):
            return hctxs[c // GROUP]

        for slot in range(n_chunks + 5):
            gens = []
            c = slot - 5
            if 0 <= c < n_chunks:
                gens.append(stage_b3(c, cxs[c]))
            c = slot - 4
            if 0 <= c < n_chunks:
                gens.append(stage_b2(c, cxs[c]))
            c = slot - 3
            if 0 <= c < n_chunks:
                gens.append(stage_b1(c, cxs[c]))
            c = slot - 2
            if 0 <= c < n_chunks:
                gens.append(stage_m(c, cxs[c], hx(c)))
            c = slot - 1
            if 0 <= c < n_chunks:
                gens.append(stage_a2(c, cxs[c]))
            c = slot
            if 0 <= c < n_chunks:
                gens.append(stage_a1(c, cxs[c]))
                if c % GROUP == 0:
                    gens.append(group_phase(c // GROUP, hctxs[c // GROUP]))
            interleave(gens)

    nc.compile()
    return nc


def _prep_consts(Wq, bq, Wk, bk, Wv, bv, Wo, bo, ln_g, ln_b):
    import ml_dtypes
    FP8 = ml_dtypes.float8_e4m3
    BF16 = ml_dtypes.bfloat16

    Wq_l = ln_g[:, None] * Wq
    bq_eff = (ln_b @ Wq + bq) * 0.125

    def wl8(w):
        return np.ascontiguousarray(
            (8.0 * w).reshape(KO, 128, D).astype(FP8))

    e2c = np.zeros((128, NP, NH), dtype=np.float32)
    for p in range(NP):
        e2c[0:64, p, 2 * p] = 1.0
        e2c[64:128, p, 2 * p + 1] = 1.0
    rpc = np.zeros((NH, NP, 128), dtype=np.float32)
    for p in range(NP):
        rpc[2 * p, p, 0:64] = 1.0
        rpc[2 * p + 1, p, 64:128] = 1.0

    return {
        "wq": wl8(Wq_l), "wk": wl8(Wk), "wv": wl8(Wv), "wo": wl8(Wo),
        "e2c": e2c.astype(BF16), "rpc": rpc.astype(BF16),
        "onesb": np.ones((1, 128), dtype=np.float32).astype(BF16),
        "bqc": np.ascontiguousarray(bq_eff.reshape(KO, 128).T, dtype=np.float32),
        "bkc": np.ascontiguousarray(bk.reshape(KO, 128).T, dtype=np.float32),
        "bvrr": np.ascontiguousarray((8.0 * bv).reshape(1, D)).astype(BF16),
        "identr": np.eye(128, dtype=np.float32),
        "ident8": np.eye(128, dtype=np.float32).astype(FP8),
    }


def kernel(e, h, Wq, bq, Wk, bk, Wv, bv, Wo, bo, ln_g, ln_b):
    from concourse.bass_utils import run_bass_kernel_spmd

    e = np.asarray(e, dtype=np.float32)
    h = np.asarray(h, dtype=np.float32)
    bo = np.asarray(bo, dtype=np.float32)
    n_chunks = e.shape[1]

    if n_chunks not in _prog_cache:
        _prog_cache[n_chunks] = _build(n_chunks)
    nc = _prog_cache[n_chunks]

    consts = _prep_consts(
        np.asarray(Wq, np.float32), np.asarray(bq, np.float32),
        np.asarray(Wk, np.float32), np.asarray(bk, np.float32),
        np.asarray(Wv, np.float32), np.asarray(bv, np.float32),
        np.asarray(Wo, np.float32), np.asarray(bo, np.float32),
        np.asarray(ln_g, np.float32), np.asarray(ln_b, np.float32))

    if np.any(bo):
        e = e + bo  # fold output bias into the residual

    in_maps = []
    for b in range(B):
        m = dict(consts)
        m["e"] = np.ascontiguousarray(e[b].reshape(n_chunks, R, D))
        m["h"] = np.ascontiguousarray(h[b])
        in_maps.append(m)

    res = run_bass_kernel_spmd(nc, in_maps, core_ids=list(range(B)))
    out = np.stack([r["out"] for r in res.results], axis=0)
    return out.reshape(B, n_chunks, N, S, D)
